# revision 1
# baseline (speedup 1.0000x reference)
"""GuidedAttention Trainium2 kernel — batch-parallel over 8 NeuronCores.

Per core (one batch element, SQ=SK=D=1024, H=16, DH=64):
  q = hs @ Wq.T + bq ; k = ctx @ Wk.T + bk ; v = ctx @ Wv.T + bv
  attn1 = softmax(q k^T / 32) ; gctx = attn1 @ v
  new_q = LN(relu(gctx @ Wobs.T + bobs)) ; new_k = LN(relu(k @ Wmat.T + bmat))
  out = MHA(new_q, new_k, v)  (16 heads of 64)

Layout strategy: activations are kept feature-major (transposed) so every
contraction has its reduced dim on SBUF partitions. hsT/ctxT/W.T are
prepared host-side. Softmaxes are computed column-wise on transposed
score matrices without max subtraction (scores are O(10), exp is safe);
the softmax sums come from a ones-augmented column of V (M=65 matmuls)
or a ones-vector matmul, and the normalization is folded downstream.
LayerNorms run in natural [s, o] layout and are PE-transposed back.
kT / new_qT / new_kT spill to DRAM between phases to fit SBUF.
Matmuls run as float32r except the probability/value path (bf16).
Output is produced transposed; the host transposes it back.
"""

import numpy as np

B, SQ, SK, D, H = 8, 1024, 1024, 1024, 16
DH = D // H
LN_EPS = 1e-5
N_CORES = 8

_CACHE = {}


def _ln_transpose_spill(nc, tmp, g_bc, b_bc, eps_t, ident, tpool, spool,
                        psum_tr, d_spill, row_idx, AF, f32, f32r):
    """LN rows of tmp [128(s), 1024(o)], apply g/b, PE-transpose the 8
    128x128 blocks and DMA the transposed [o, s] stripe to d_spill."""
    stats = tpool.tile([128, 2, 6], f32, tag="stats")
    mv = tpool.tile([128, 2], f32, tag="mv")
    for sg in range(2):
        nc.vector.bn_stats(stats[:, sg, :], tmp[:, sg * 512:(sg + 1) * 512])
    nc.vector.bn_aggr(mv[:], stats[:])
    lnv = tpool.tile([128, 1], f32, tag="lnv")
    rstd = tpool.tile([128, 1], f32, tag="rstd")
    nc.scalar.activation(lnv[:], mv[:, 1:2], AF.Ln, bias=eps_t[:])
    nc.scalar.activation(rstd[:], lnv[:], AF.Exp, scale=-0.5)
    negmr = tpool.tile([128, 1], f32, tag="negmr")
    nc.vector.tensor_mul(negmr[:], mv[:, 0:1], rstd[:])
    nc.vector.tensor_scalar_mul(negmr[:], negmr[:], -1.0)
    import concourse.mybir as mybir
    nc.vector.tensor_scalar(
        tmp[:], tmp[:], rstd[:], negmr[:],
        mybir.AluOpType.mult, mybir.AluOpType.add,
    )
    nc.vector.tensor_mul(tmp[:], tmp[:], g_bc[:])
    nc.vector.tensor_add(tmp[:], tmp[:], b_bc[:])
    stg = spool.tile([128, 8, 128], f32r, tag="tstage")
    for ot in range(8):
        pst = psum_tr.tile([128, 128], f32, tag="tr")
        nc.tensor.transpose(pst[:], tmp[:, ot * 128:(ot + 1) * 128], ident[:])
        nc.vector.tensor_copy(stg[:, ot, :], pst[:])
    nc.sync.dma_start(
        d_spill.rearrange("(c p) s -> p c s", p=128)[
            :, :, row_idx * 128:(row_idx + 1) * 128
        ],
        stg.bitcast(f32),
    )


def _build():
    import concourse.mybir as mybir
    import concourse.tile as tile
    from concourse import bacc
    from concourse.masks import make_identity

    f32 = mybir.dt.float32
    f32r = mybir.dt.float32r
    bf16 = mybir.dt.bfloat16
    AF = mybir.ActivationFunctionType

    nc = bacc.Bacc(None, target_bir_lowering=False)

    d_hsT = nc.dram_tensor("hsT", [D, SQ], f32, kind="ExternalInput")
    d_ctxT = nc.dram_tensor("ctxT", [D, SK], f32, kind="ExternalInput")
    d_w = {
        n: nc.dram_tensor(n, [D, D], f32, kind="ExternalInput")
        for n in ("WqT", "WkT", "WvT", "WobsT", "WmatT")
    }
    d_b = {
        n: nc.dram_tensor(n, [D], f32, kind="ExternalInput")
        for n in ("bq", "bk", "bv", "bobs", "bmat", "ln_g", "ln_b")
    }
    d_outT = nc.dram_tensor("outT", [D, SQ], f32, kind="ExternalOutput")
    d_kT = nc.dram_tensor("kT_spill", [D, SK], f32, kind="Internal")
    d_nq = nc.dram_tensor("nq_spill", [D, SQ], f32, kind="Internal")
    d_nk = nc.dram_tensor("nk_spill", [D, SK], f32, kind="Internal")
    d_rs = nc.dram_tensor("rs_scratch", [SQ], f32, kind="Internal")
    d_rs2 = nc.dram_tensor("rs2_scratch", [16, SQ], f32, kind="Internal")

    def r8(ap):  # [(c p), x] -> [p, c, x]
        return ap.rearrange("(c p) x -> p c x", p=128)

    def vec2d(name):  # [D] -> [1, D] AP
        return d_b[name][:].rearrange("(a d) -> a d", a=1)

    with tile.TileContext(nc) as tc:
        with tc.tile_pool(name="persist", bufs=1) as pp:
            v_aug = pp.tile([128, 8, H, DH + 1], bf16, tag="vaug")
            v_plain = pp.tile([128, 8, D], bf16, tag="vplain")
            ident = pp.tile([128, 128], f32, tag="ident")
            make_identity(nc, ident[:])
            ones_row = pp.tile([1, 128], f32r, tag="ones_row")
            d_ones = nc.inline_tensor(np.ones((1, 128), np.float32), name="ones_const")
            nc.sync.dma_start(ones_row[:], d_ones[:].bitcast(f32r))
            ones_col_bf = pp.tile([128, 1], bf16, tag="ones_col")
            nc.vector.memset(ones_col_bf[:], 1.0)
            eps_t = pp.tile([128, 1], f32, tag="eps")
            nc.vector.memset(eps_t[:], LN_EPS)
            bq_sc = pp.tile([128, 8], f32, tag="bq_sc")
            bk_sc = pp.tile([128, 8], f32, tag="bk_sc")
            nc.sync.dma_start(bq_sc[:], d_b["bq"][:].rearrange("(c p) -> p c", p=128))
            nc.sync.dma_start(bk_sc[:], d_b["bk"][:].rearrange("(c p) -> p c", p=128))
            rS_sc = pp.tile([128, 8], f32, tag="rS_sc")
            bobs_row = pp.tile([1, D], f32r, tag="bobs_row")
            nc.sync.dma_start(bobs_row[:], vec2d("bobs").bitcast(f32r))
            S_row = pp.tile([1, SQ], f32r, tag="S_row")
            g_bc = pp.tile([128, D], f32, tag="g_bc")
            b_bc = pp.tile([128, D], f32, tag="b_bc")
            nc.sync.dma_start(g_bc[:], vec2d("ln_g").to_broadcast([128, D]))
            nc.sync.dma_start(b_bc[:], vec2d("ln_b").to_broadcast([128, D]))

            # ============ phase A: kT, v_aug, new_kT ============
            with (
                tc.tile_pool(name="pa", bufs=1) as pa,
                tc.tile_pool(name="pa_ctx", bufs=2) as pactx,
                tc.tile_pool(name="pa_wk", bufs=2) as pawk,
                tc.tile_pool(name="pa_wf", bufs=1) as pawf,
                tc.tile_pool(name="pa_tmp", bufs=2) as pat,
                tc.tile_pool(name="pa_st", bufs=1) as pas,
                tc.tile_pool(name="pa_ps", bufs=4, space="PSUM") as pap,
                tc.tile_pool(name="pa_ptr", bufs=2, space="PSUM") as papt,
            ):
                kT = pa.tile([128, 8, SK], f32r, tag="kT")  # [o_p, o_c, t]
                bv_bc = pa.tile([128, D], f32, tag="bv_bc")
                nc.sync.dma_start(bv_bc[:], vec2d("bv").to_broadcast([128, D]))
                bmat_row = pa.tile([1, D], f32r, tag="bmat_row")
                nc.sync.dma_start(bmat_row[:], vec2d("bmat").bitcast(f32r))
                nc.gpsimd.memset(v_aug[:, :, :, DH:DH + 1], 1.0)
                wv_f = pawf.tile([128, 8, D], f32r, tag="wfull")
                nc.sync.dma_start(wv_f[:], r8(d_w["WvT"]).bitcast(f32r))

                for tch in range(2):
                    tcs = slice(tch * 512, (tch + 1) * 512)
                    ctx_c = pactx.tile([128, 8, 512], f32r, tag="ctx")
                    nc.sync.dma_start(ctx_c[:], r8(d_ctxT)[:, :, tcs].bitcast(f32r))
                    # kT[o, tch] = WkT.T @ ctxT + bk
                    for ot in range(8):
                        wk_c = pawk.tile([128, 8, 128], f32r, tag="wk")
                        nc.sync.dma_start(
                            wk_c[:],
                            r8(d_w["WkT"])[:, :, ot * 128:(ot + 1) * 128].bitcast(f32r),
                        )
                        ps = pap.tile([128, 512], f32, tag="mm")
                        for i in range(8):
                            nc.tensor.matmul(
                                ps[:], wk_c[:, i, :], ctx_c[:, i, :],
                                start=(i == 0), stop=(i == 7),
                            )
                        nc.vector.tensor_scalar_add(
                            kT[:, ot, tcs], ps[:], bk_sc[:, ot:ot + 1]
                        )
                    # v rows of this t-chunk
                    for oc in range(2):
                        for tl in range(4):
                            tt = tch * 4 + tl
                            ps = pap.tile([128, 512], f32, tag="mm")
                            for i in range(8):
                                nc.tensor.matmul(
                                    ps[:], ctx_c[:, i, tl * 128:(tl + 1) * 128],
                                    wv_f[:, i, oc * 512:(oc + 1) * 512],
                                    start=(i == 0), stop=(i == 7),
                                )
                            nc.vector.tensor_add(
                                v_aug[:, tt, oc * 8:(oc + 1) * 8, 0:DH],
                                ps[:].rearrange("p (h j) -> p h j", j=DH),
                                bv_bc[:, oc * 512:(oc + 1) * 512].rearrange(
                                    "p (h j) -> p h j", j=DH
                                ),
                            )
                            nc.vector.tensor_add(
                                v_plain[:, tt, oc * 512:(oc + 1) * 512],
                                ps[:],
                                bv_bc[:, oc * 512:(oc + 1) * 512],
                            )
                nc.sync.dma_start(r8(d_kT), kT.bitcast(f32))

                # new_k: relu(k @ Wmat.T + bmat) -> LN -> transpose -> spill
                wm_f = pawf.tile([128, 8, D], f32r, tag="wfull")
                nc.sync.dma_start(wm_f[:], r8(d_w["WmatT"]).bitcast(f32r))
                for tt in range(8):
                    tmp = pat.tile([128, 1024], f32, tag="tmpk")
                    for oc in range(2):
                        ps = pap.tile([128, 512], f32, tag="mm")
                        for i in range(8):
                            nc.tensor.matmul(
                                ps[:], kT[:, i, tt * 128:(tt + 1) * 128],
                                wm_f[:, i, oc * 512:(oc + 1) * 512],
                                start=(i == 0), stop=False,
                            )
                        nc.tensor.matmul(
                            ps[:], ones_row[:], bmat_row[:, oc * 512:(oc + 1) * 512],
                            start=False, stop=True,
                        )
                        nc.scalar.activation(
                            tmp[:, oc * 512:(oc + 1) * 512], ps[:], AF.Relu
                        )
                    _ln_transpose_spill(nc, tmp, g_bc, b_bc, eps_t, ident, pat,
                                        pas, papt, d_nk, tt, AF, f32, f32r)

            # ============ phase B: q path ============
            with (
                tc.tile_pool(name="pb", bufs=1) as pb,
                tc.tile_pool(name="pb_wq", bufs=2) as pbwq,
                tc.tile_pool(name="pb_wo", bufs=1) as pbwo,
                tc.tile_pool(name="pb_tmp", bufs=2) as pbt,
                tc.tile_pool(name="pb_st", bufs=1) as pbs,
                tc.tile_pool(name="pb_ps", bufs=4, space="PSUM") as pbp,
                tc.tile_pool(name="pb_ptr", bufs=2, space="PSUM") as pbpt,
                tc.tile_pool(name="pb_psS", bufs=2, space="PSUM") as pbpS,
            ):
                kTb = pb.tile([128, 8, SK], f32r, tag="kTb")
                nc.sync.dma_start(kTb[:], r8(d_kT).bitcast(f32r))
                wobs_f = pbwo.tile([128, 8, D], f32r, tag="wobs")
                nc.sync.dma_start(wobs_f[:], r8(d_w["WobsT"]).bitcast(f32r))
                rS_row = pb.tile([1, SQ], f32, tag="rS_row")

                for sc in range(2):
                    scs = slice(sc * 512, (sc + 1) * 512)
                    hs_c = pb.tile([128, 8, 512], f32r, tag="hs")
                    nc.sync.dma_start(hs_c[:], r8(d_hsT)[:, :, scs].bitcast(f32r))
                    qT_c = pb.tile([128, 8, 512], f32r, tag="qs")
                    for ot in range(8):
                        wq_c = pbwq.tile([128, 8, 128], f32r, tag="wq")
                        nc.sync.dma_start(
                            wq_c[:],
                            r8(d_w["WqT"])[:, :, ot * 128:(ot + 1) * 128].bitcast(f32r),
                        )
                        ps = pbp.tile([128, 512], f32, tag="mm")
                        for i in range(8):
                            nc.tensor.matmul(
                                ps[:], wq_c[:, i, :], hs_c[:, i, :],
                                start=(i == 0), stop=(i == 7),
                            )
                        nc.vector.tensor_scalar_add(
                            qT_c[:, ot, :], ps[:], bq_sc[:, ot:ot + 1]
                        )
                    # probsT = exp(scoresT / 32), bf16
                    probs = pb.tile([128, 8, 512], bf16, tag="probs")
                    for tt in range(8):
                        ps = pbp.tile([128, 512], f32, tag="mm")
                        for oc in range(8):
                            nc.tensor.matmul(
                                ps[:], kTb[:, oc, tt * 128:(tt + 1) * 128],
                                qT_c[:, oc, :],
                                start=(oc == 0), stop=(oc == 7),
                            )
                        nc.scalar.activation(
                            probs[:, tt, :], ps[:], AF.Exp, scale=1.0 / 32.0
                        )
                    # column sums + reciprocal, scattered to [s_p, s_c]
                    psS = pbpS.tile([1, 512], f32, tag="sS")
                    for tt in range(8):
                        nc.tensor.matmul(
                            psS[:], ones_col_bf[:], probs[:, tt, :],
                            start=(tt == 0), stop=(tt == 7),
                        )
                    nc.vector.tensor_copy(S_row[:, scs], psS[:])
                    nc.vector.reciprocal(rS_row[:, scs], psS[:])
                    nc.sync.dma_start(d_rs[scs], rS_row[:, scs])
                    nc.sync.dma_start(
                        rS_sc[:, sc * 4:(sc + 1) * 4],
                        d_rs[scs].rearrange("(c p) -> p c", p=128),
                    )
                    # gctxT[o, s] = v.T @ probsT  (unnormalized)
                    gctx = pb.tile([128, 8, 512], f32r, tag="qs")
                    for ot in range(8):
                        ps = pbp.tile([128, 512], f32, tag="mm")
                        for tt in range(8):
                            nc.tensor.matmul(
                                ps[:], v_plain[:, tt, ot * 128:(ot + 1) * 128],
                                probs[:, tt, :],
                                start=(tt == 0), stop=(tt == 7),
                            )
                        nc.vector.tensor_copy(gctx[:, ot, :], ps[:])
                    # preq: relu((gctx_raw @ WobsT + S*bobs) / S) -> LN -> T
                    for st in range(4):
                        gst = sc * 4 + st
                        tmp = pbt.tile([128, 1024], f32, tag="tmpk")
                        for oc in range(2):
                            ps = pbp.tile([128, 512], f32, tag="mm")
                            for i in range(8):
                                nc.tensor.matmul(
                                    ps[:], gctx[:, i, st * 128:(st + 1) * 128],
                                    wobs_f[:, i, oc * 512:(oc + 1) * 512],
                                    start=(i == 0), stop=False,
                                )
                            nc.tensor.matmul(
                                ps[:], S_row[:, gst * 128:(gst + 1) * 128],
                                bobs_row[:, oc * 512:(oc + 1) * 512],
                                start=False, stop=True,
                            )
                            nc.scalar.activation(
                                tmp[:, oc * 512:(oc + 1) * 512], ps[:], AF.Relu,
                                scale=rS_sc[:, gst:gst + 1],
                            )
                        _ln_transpose_spill(nc, tmp, g_bc, b_bc, eps_t, ident,
                                            pbt, pbs, pbpt, d_nq, gst, AF, f32, f32r)

            # ============ phase D: 16-head MHA ============
            with (
                tc.tile_pool(name="pd", bufs=1) as pd,
                tc.tile_pool(name="pd_probs", bufs=3) as pdp,
                tc.tile_pool(name="pd_st", bufs=3) as pds,
                tc.tile_pool(name="pd_ps", bufs=2, space="PSUM") as pdps,
                tc.tile_pool(name="pd_po", bufs=2, space="PSUM") as pdpo,
            ):
                nqT = pd.tile([128, 8, SQ], f32r, tag="nqT")
                nkT = pd.tile([128, 8, SK], f32r, tag="nkT")
                nc.sync.dma_start(nqT[:], r8(d_nq).bitcast(f32r))
                nc.sync.dma_start(nkT[:], r8(d_nk).bitcast(f32r))
                def emit_scores(h, probs):
                    po = (h % 2) * 64
                    hc = h // 2
                    for tt in range(8):
                        ps = pdps.tile([128, 1024], f32, tag="psc")
                        for sh in range(2):
                            nc.tensor.matmul(
                                ps[:, sh * 512:(sh + 1) * 512],
                                nkT[po:po + 64, hc, tt * 128:(tt + 1) * 128],
                                nqT[po:po + 64, hc, sh * 512:(sh + 1) * 512],
                                start=True, stop=True,
                            )
                        nc.scalar.activation(
                            probs[:, tt, :], ps[:], AF.Exp, scale=1.0 / 8.0
                        )

                def emit_out(h, probs):
                    stage = pds.tile([65, SQ], f32, tag="stage")
                    for sh in range(2):
                        ps = pdpo.tile([65, 1024], f32, tag="po")
                        for tt in range(8):
                            nc.tensor.matmul(
                                ps[:, sh * 512:(sh + 1) * 512],
                                v_aug[:, tt, h, :],
                                probs[:, tt, sh * 512:(sh + 1) * 512],
                                start=(tt == 0), stop=(tt == 7),
                            )
                        nc.vector.tensor_copy(
                            stage[:, sh * 512:(sh + 1) * 512],
                            ps[:, sh * 512:(sh + 1) * 512],
                        )
                    nc.vector.reciprocal(stage[64:65, :], stage[64:65, :])
                    nc.sync.dma_start(d_rs2[h:h + 1, :], stage[64:65, :])
                    rbc = pds.tile([64, SQ], f32, tag="rbc")
                    nc.sync.dma_start(
                        rbc[:], d_rs2[h:h + 1, :].to_broadcast([64, SQ])
                    )
                    outF = pds.tile([64, SQ], f32, tag="outF")
                    nc.vector.tensor_mul(outF[:], stage[0:64, :], rbc[:])
                    nc.sync.dma_start(d_outT[h * DH:(h + 1) * DH, :], outF[:])

                for hp in range(H // 2):
                    h0, h1 = 2 * hp, 2 * hp + 1
                    probs0 = pdp.tile([128, 8, SQ], bf16, tag="probs_h")
                    emit_scores(h0, probs0)
                    probs1 = pdp.tile([128, 8, SQ], bf16, tag="probs_h")
                    emit_scores(h1, probs1)
                    emit_out(h0, probs0)
                    emit_out(h1, probs1)

    nc.compile()
    return nc


def kernel(hidden_states, context, Wq, bq, Wk, bk, Wv, bv,
           Wobs, bobs, Wmat, bmat, ln_g, ln_b):
    from concourse import bass_utils

    if "nc" not in _CACHE:
        _CACHE["nc"] = _build()
    nc = _CACHE["nc"]

    w = {
        "WqT": np.ascontiguousarray(np.asarray(Wq).T),
        "WkT": np.ascontiguousarray(np.asarray(Wk).T),
        "WvT": np.ascontiguousarray(np.asarray(Wv).T),
        "WobsT": np.ascontiguousarray(np.asarray(Wobs).T),
        "WmatT": np.ascontiguousarray(np.asarray(Wmat).T),
    }
    vecs = {"bq": bq, "bk": bk, "bv": bv, "bobs": bobs, "bmat": bmat,
            "ln_g": ln_g, "ln_b": ln_b}
    in_maps = []
    for b in range(N_CORES):
        m = {"hsT": np.ascontiguousarray(np.asarray(hidden_states[b]).T),
             "ctxT": np.ascontiguousarray(np.asarray(context[b]).T)}
        m.update(w)
        m.update({k: np.ascontiguousarray(np.asarray(v)) for k, v in vecs.items()})
        in_maps.append(m)

    res = bass_utils.run_bass_kernel_spmd(nc, in_maps, core_ids=list(range(N_CORES)))
    out = np.stack([res.results[b]["outT"].T for b in range(N_CORES)], axis=0)
    return out.astype(np.float32)



# revision 10
# speedup vs baseline: 1.6321x; 1.6321x over previous
"""GuidedAttention Trainium2 kernel — batch-parallel over 8 NeuronCores.

Per core (one batch element, SQ=SK=D=1024, H=16, DH=64):
  q = hs @ Wq.T + bq ; k = ctx @ Wk.T + bk ; v = ctx @ Wv.T + bv
  attn1 = softmax(q k^T / 32) ; gctx = attn1 @ v
  new_q = LN(relu(gctx @ Wobs.T + bobs)) ; new_k = LN(relu(k @ Wmat.T + bmat))
  out = MHA(new_q, new_k, v)  (16 heads of 64)

v2 design (vs the spill-based f32r baseline):
  * all matmul operands are bf16 (fp32 PSUM accumulation); host converts.
  * every intermediate (kT, v, new_qT, new_kT) stays resident in SBUF as
    bf16 — no DRAM spills.
  * guide softmax normalization is folded into the LayerNorm using LN's
    scale invariance: LN of the UNnormalized rows with eps' = eps * S^2
    is exactly LN of the normalized rows (the S*bobs rank-1 bias matmul
    keeps the bias consistent). No reciprocal needed at all in phase B.
  * LN statistics are batched across stripes so the Sqrt activation runs
    once per group (avoids activation-table thrashing), rstd via exact
    DVE reciprocal on a [128, n] tile.
  * MHA head pairs run row-packed (DH=64 contraction on array rows 0-63 /
    64-127 concurrently via tile_position auto-derivation). Softmax sums
    come free from a ones-column in v_aug; 1/S via reciprocal_approx_fast;
    the [64, SQ] broadcast of 1/S is a rank-1 PE matmul (no DRAM bounce).
Output is produced transposed [D, SQ]; the host transposes it back.
"""

import numpy as np

B, SQ, SK, D, H = 8, 1024, 1024, 1024, 16
DH = D // H
LN_EPS = 1e-5
N_CORES = 8

_CACHE = {}


def _build():
    import concourse.mybir as mybir
    import concourse.tile as tile
    from concourse import bacc
    from concourse.masks import make_identity

    f32 = mybir.dt.float32
    f32r = mybir.dt.float32r
    bf16 = mybir.dt.bfloat16
    AF = mybir.ActivationFunctionType
    ALU = mybir.AluOpType

    nc = bacc.Bacc(None, target_bir_lowering=False)

    d_hsT = nc.dram_tensor("hsT", [D, SQ], bf16, kind="ExternalInput")
    d_ctxT = nc.dram_tensor("ctxT", [D, SK], bf16, kind="ExternalInput")
    d_w = {
        n: nc.dram_tensor(n, [D, D], bf16, kind="ExternalInput")
        for n in ("WqT", "WkT", "WvT", "WobsT", "WmatT")
    }
    d_b = {
        n: nc.dram_tensor(n, [D], f32, kind="ExternalInput")
        for n in ("bq", "bk", "bv", "bobs", "bmat")
    }
    d_gbf = nc.dram_tensor("ln_g_bf", [D], bf16, kind="ExternalInput")
    d_bbf = nc.dram_tensor("ln_b_bf", [D], bf16, kind="ExternalInput")
    d_outT = nc.dram_tensor("outT", [D, SQ], f32, kind="ExternalOutput")
    d_rs = nc.dram_tensor("rs_scratch", [SQ], f32, kind="Internal")
    d_rs2 = nc.dram_tensor("rs2_scratch", [16, SQ], f32, kind="Internal")

    def r8(ap):  # [(c p), x] -> [p, c, x]
        return ap.rearrange("(c p) x -> p c x", p=128)

    def vec2d(name):  # [D] -> [1, D] AP
        return d_b[name][:].rearrange("(a d) -> a d", a=1)

    with tile.TileContext(nc) as tc:
        with tc.tile_pool(name="persist", bufs=1) as pp:
            # ---- constants / persistent tiles ----
            v_aug = pp.tile([128, 8, H, DH + 1], bf16, tag="vaug")
            v_plain = pp.tile([128, 8, D], bf16, tag="vplain")
            nqT = pp.tile([128, 8, SQ], bf16, tag="nqT")
            nkT = pp.tile([128, 8, SK], bf16, tag="nkT")
            ident_bf = pp.tile([128, 128], bf16, tag="ident_bf")
            make_identity(nc, ident_bf[:])
            ones_row = pp.tile([1, 128], f32r, tag="ones_row")
            d_ones = nc.inline_tensor(np.ones((1, 128), np.float32), name="ones_const")
            nc.sync.dma_start(ones_row[:], d_ones[:].bitcast(f32r))
            ones_col_bf = pp.tile([128, 1], bf16, tag="ones_col")
            nc.vector.memset(ones_col_bf[:], 1.0)
            ones_row_bf = pp.tile([1, 128], bf16, tag="ones_row_bf")
            nc.vector.memset(ones_row_bf[:], 1.0)
            bq_sc = pp.tile([128, 8], f32, tag="bq_sc")
            bk_sc = pp.tile([128, 8], f32, tag="bk_sc")
            nc.sync.dma_start(bq_sc[:], d_b["bq"][:].rearrange("(c p) -> p c", p=128))
            nc.sync.dma_start(bk_sc[:], d_b["bk"][:].rearrange("(c p) -> p c", p=128))
            bobs_row = pp.tile([1, D], f32r, tag="bobs_row")
            nc.sync.dma_start(bobs_row[:], vec2d("bobs").bitcast(f32r))
            g_bc = pp.tile([128, D], bf16, tag="g_bc")
            b_bc = pp.tile([128, D], bf16, tag="b_bc")
            nc.sync.dma_start(
                g_bc[:], d_gbf[:].rearrange("(a d) -> a d", a=1).to_broadcast([128, D])
            )
            nc.sync.dma_start(
                b_bc[:], d_bbf[:].rearrange("(a d) -> a d", a=1).to_broadcast([128, D])
            )
            # guide softmax column sums, [s_p, s_c] layout + derived eps'
            Sc = pp.tile([128, 8], f32, tag="Sc")
            eps_q = pp.tile([128, 8], f32, tag="eps_q")
            S_row = pp.tile([1, SQ], f32r, tag="S_row")

            def ln_batch_apply(tmp, n_str, var_all, mean_all, eps_col, tpool,
                               psum_tr, dst, dst_off):
                """Given tmp [128, n_str, 1024] bf16 rows (s on partitions) and
                per-stripe stats APs [128, n_str], LN+affine each stripe, then
                PE-transpose 128x128 blocks into dst[:, ot, dst_off + i*128]."""
                lnv = tpool.tile([128, n_str], f32, tag="lnv")
                nc.vector.tensor_tensor(lnv[:], var_all, eps_col, ALU.add)
                std = tpool.tile([128, n_str], f32, tag="std")
                nc.scalar.activation(std[:], lnv[:], AF.Sqrt)
                rstd = tpool.tile([128, n_str], f32, tag="rstd")
                nc.vector.reciprocal(rstd[:], std[:])
                negmr = tpool.tile([128, n_str], f32, tag="negmr")
                nc.vector.tensor_mul(negmr[:], mean_all, rstd[:])
                nc.vector.tensor_scalar_mul(negmr[:], negmr[:], -1.0)
                for i in range(n_str):
                    nc.vector.tensor_scalar(
                        tmp[:, i, :], tmp[:, i, :],
                        rstd[:, i:i + 1], negmr[:, i:i + 1],
                        ALU.mult, ALU.add,
                    )
                    nc.vector.tensor_mul(tmp[:, i, :], tmp[:, i, :], g_bc[:])
                    nc.vector.tensor_add(tmp[:, i, :], tmp[:, i, :], b_bc[:])
                    for ot in range(8):
                        pst = psum_tr.tile([128, 128], bf16, tag="tr")
                        nc.tensor.transpose(
                            pst[:], tmp[:, i, ot * 128:(ot + 1) * 128], ident_bf[:]
                        )
                        nc.vector.tensor_copy(
                            dst[:, ot, dst_off + i * 128:dst_off + (i + 1) * 128],
                            pst[:],
                        )

            # ============ phases A+B share kT ============
            pab_cm = tc.tile_pool(name="pab", bufs=1)
            pab = pab_cm.__enter__()
            kT = pab.tile([128, 8, SK], bf16, tag="kT")

            # ============ phase A: kT, v, new_kT ============
            with (
                tc.tile_pool(name="pa", bufs=1) as pa,
                tc.tile_pool(name="pa_ctx", bufs=2) as pactx,
                tc.tile_pool(name="pa_wk", bufs=2) as pawk,
                tc.tile_pool(name="pa_ps", bufs=4, space="PSUM") as pap,
                tc.tile_pool(name="pa_ptr", bufs=2, space="PSUM") as papt,
            ):
                bv_bc = pa.tile([128, D], f32, tag="bv_bc")
                nc.sync.dma_start(bv_bc[:], vec2d("bv").to_broadcast([128, D]))
                bmat_row = pa.tile([1, D], f32r, tag="bmat_row")
                nc.sync.dma_start(bmat_row[:], vec2d("bmat").bitcast(f32r))
                nc.gpsimd.memset(v_aug[:, :, :, DH:DH + 1], 1.0)
                wv_f = pa.tile([128, 8, D], bf16, tag="wv")
                nc.sync.dma_start(wv_f[:], r8(d_w["WvT"]))
                wm_f = pa.tile([128, 8, D], bf16, tag="wm")
                nc.sync.dma_start(wm_f[:], r8(d_w["WmatT"]))

                for tch in range(2):
                    tcs = slice(tch * 512, (tch + 1) * 512)
                    ctx_c = pactx.tile([128, 8, 512], bf16, tag="ctx")
                    nc.sync.dma_start(ctx_c[:], r8(d_ctxT)[:, :, tcs])
                    # kT[o, tch] = WkT.T @ ctxT + bk
                    for ot in range(8):
                        wk_c = pawk.tile([128, 8, 128], bf16, tag="wk")
                        nc.sync.dma_start(
                            wk_c[:], r8(d_w["WkT"])[:, :, ot * 128:(ot + 1) * 128]
                        )
                        ps = pap.tile([128, 512], f32, tag="mm")
                        for i in range(8):
                            nc.tensor.matmul(
                                ps[:], wk_c[:, i, :], ctx_c[:, i, :],
                                start=(i == 0), stop=(i == 7),
                            )
                        nc.vector.tensor_scalar_add(
                            kT[:, ot, tcs], ps[:], bk_sc[:, ot:ot + 1]
                        )
                    # v rows of this t-chunk
                    for oc in range(2):
                        for tl in range(4):
                            tt = tch * 4 + tl
                            ps = pap.tile([128, 512], f32, tag="mm")
                            for i in range(8):
                                nc.tensor.matmul(
                                    ps[:], ctx_c[:, i, tl * 128:(tl + 1) * 128],
                                    wv_f[:, i, oc * 512:(oc + 1) * 512],
                                    start=(i == 0), stop=(i == 7),
                                )
                            nc.vector.tensor_add(
                                v_aug[:, tt, oc * 8:(oc + 1) * 8, 0:DH],
                                ps[:].rearrange("p (h j) -> p h j", j=DH),
                                bv_bc[:, oc * 512:(oc + 1) * 512].rearrange(
                                    "p (h j) -> p h j", j=DH
                                ),
                            )
                            nc.vector.tensor_add(
                                v_plain[:, tt, oc * 512:(oc + 1) * 512],
                                ps[:],
                                bv_bc[:, oc * 512:(oc + 1) * 512],
                            )

                # new_k: relu(k @ Wmat.T + bmat) -> LN -> transpose -> nkT
                tmpk = pa.tile([128, 8, 1024], bf16, tag="tmpk")
                stats = pa.tile([128, 8, 2, 6], f32, tag="stats")
                mv = pa.tile([128, 8, 2], f32, tag="mv")
                for tt in range(8):
                    for oc in range(2):
                        ps = pap.tile([128, 512], f32, tag="mm")
                        for i in range(8):
                            nc.tensor.matmul(
                                ps[:], kT[:, i, tt * 128:(tt + 1) * 128],
                                wm_f[:, i, oc * 512:(oc + 1) * 512],
                                start=(i == 0), stop=False,
                            )
                        nc.tensor.matmul(
                            ps[:], ones_row[:], bmat_row[:, oc * 512:(oc + 1) * 512],
                            start=False, stop=True,
                        )
                        nc.scalar.activation(
                            tmpk[:, tt, oc * 512:(oc + 1) * 512], ps[:], AF.Relu
                        )
                        nc.vector.bn_stats(
                            stats[:, tt, oc, :], tmpk[:, tt, oc * 512:(oc + 1) * 512]
                        )
                    nc.vector.bn_aggr(mv[:, tt, :], stats[:, tt, :, :])
                eps_k = pa.tile([128, 8], f32, tag="eps_k")
                nc.vector.memset(eps_k[:], LN_EPS)
                ln_batch_apply(tmpk, 8, mv[:, :, 1], mv[:, :, 0], eps_k[:], pa,
                               papt, nkT, 0)

            # ============ phase B: q path -> nqT ============
            with (
                tc.tile_pool(name="pb", bufs=1) as pb,
                tc.tile_pool(name="pb_hs", bufs=2) as pbhs,
                tc.tile_pool(name="pb_wq", bufs=2) as pbwq,
                tc.tile_pool(name="pb_probs", bufs=2) as pbpr,
                tc.tile_pool(name="pb_tmp", bufs=2) as pbt,
                tc.tile_pool(name="pb_ps", bufs=4, space="PSUM") as pbp,
                tc.tile_pool(name="pb_ptr", bufs=2, space="PSUM") as pbpt,
                tc.tile_pool(name="pb_psS", bufs=2, space="PSUM") as pbpS,
            ):
                wobs_f = pb.tile([128, 8, D], bf16, tag="wobs")
                nc.sync.dma_start(wobs_f[:], r8(d_w["WobsT"]))

                for sc in range(2):
                    scs = slice(sc * 512, (sc + 1) * 512)
                    hs_c = pbhs.tile([128, 8, 512], bf16, tag="hs")
                    nc.sync.dma_start(hs_c[:], r8(d_hsT)[:, :, scs])
                    qT_c = pbhs.tile([128, 8, 512], bf16, tag="qs")
                    for ot in range(8):
                        wq_c = pbwq.tile([128, 8, 128], bf16, tag="wq")
                        nc.sync.dma_start(
                            wq_c[:], r8(d_w["WqT"])[:, :, ot * 128:(ot + 1) * 128]
                        )
                        ps = pbp.tile([128, 512], f32, tag="mm")
                        for i in range(8):
                            nc.tensor.matmul(
                                ps[:], wq_c[:, i, :], hs_c[:, i, :],
                                start=(i == 0), stop=(i == 7),
                            )
                        nc.vector.tensor_scalar_add(
                            qT_c[:, ot, :], ps[:], bq_sc[:, ot:ot + 1]
                        )
                    # probsT = exp(scoresT / 32), bf16 (unnormalized)
                    probs = pbpr.tile([128, 8, 512], bf16, tag="probs")
                    for tt in range(8):
                        ps = pbp.tile([128, 512], f32, tag="mm")
                        for oc in range(8):
                            nc.tensor.matmul(
                                ps[:], kT[:, oc, tt * 128:(tt + 1) * 128],
                                qT_c[:, oc, :],
                                start=(oc == 0), stop=(oc == 7),
                            )
                        nc.scalar.activation(
                            probs[:, tt, :], ps[:], AF.Exp, scale=1.0 / 32.0
                        )
                    # column sums S[s]; spread to [s_p, s_c] via DRAM bounce
                    psS = pbpS.tile([1, 512], f32, tag="sS")
                    for tt in range(8):
                        nc.tensor.matmul(
                            psS[:], ones_col_bf[:], probs[:, tt, :],
                            start=(tt == 0), stop=(tt == 7),
                        )
                    nc.vector.tensor_copy(S_row[:, scs], psS[:])
                    nc.sync.dma_start(d_rs[scs], S_row[:, scs].bitcast(f32))
                    nc.sync.dma_start(
                        Sc[:, sc * 4:(sc + 1) * 4],
                        d_rs[scs].rearrange("(c p) -> p c", p=128),
                    )
                    # gctxT[o, s] = v.T @ probsT  (unnormalized)
                    gctx = pbhs.tile([128, 8, 512], bf16, tag="qs")
                    for ot in range(8):
                        ps = pbp.tile([128, 512], f32, tag="mm")
                        for tt in range(8):
                            nc.tensor.matmul(
                                ps[:], v_plain[:, tt, ot * 128:(ot + 1) * 128],
                                probs[:, tt, :],
                                start=(tt == 0), stop=(tt == 7),
                            )
                        nc.vector.tensor_copy(gctx[:, ot, :], ps[:])
                    # preq = relu(gctx_raw @ WobsT + S*bobs); LN w/ eps*S^2
                    tmpq = pbt.tile([128, 4, 1024], bf16, tag="tmpq")
                    statq = pbt.tile([128, 4, 2, 6], f32, tag="statq")
                    mvq = pbt.tile([128, 4, 2], f32, tag="mvq")
                    for st in range(4):
                        gst = sc * 4 + st
                        for oc in range(2):
                            ps = pbp.tile([128, 512], f32, tag="mm")
                            for i in range(8):
                                nc.tensor.matmul(
                                    ps[:], gctx[:, i, st * 128:(st + 1) * 128],
                                    wobs_f[:, i, oc * 512:(oc + 1) * 512],
                                    start=(i == 0), stop=False,
                                )
                            nc.tensor.matmul(
                                ps[:], S_row[:, gst * 128:(gst + 1) * 128],
                                bobs_row[:, oc * 512:(oc + 1) * 512],
                                start=False, stop=True,
                            )
                            nc.scalar.activation(
                                tmpq[:, st, oc * 512:(oc + 1) * 512], ps[:], AF.Relu
                            )
                            nc.vector.bn_stats(
                                statq[:, st, oc, :],
                                tmpq[:, st, oc * 512:(oc + 1) * 512],
                            )
                        nc.vector.bn_aggr(mvq[:, st, :], statq[:, st, :, :])
                    # eps' = eps * S^2 for this half
                    ecols = Sc[:, sc * 4:(sc + 1) * 4]
                    eq = eps_q[:, sc * 4:(sc + 1) * 4]
                    nc.vector.tensor_mul(eq, ecols, ecols)
                    nc.vector.tensor_scalar_mul(eq, eq, LN_EPS)
                    ln_batch_apply(tmpq, 4, mvq[:, :, 1], mvq[:, :, 0], eq, pbt,
                                   pbpt, nqT, sc * 512)
            pab_cm.__exit__(None, None, None)

            # ============ phase D: 16-head MHA ============
            with (
                tc.tile_pool(name="pd", bufs=1) as pd,
                tc.tile_pool(name="pd_probs", bufs=3) as pdp,
                tc.tile_pool(name="pd_st", bufs=3) as pds,
                tc.tile_pool(name="pd_ps", bufs=2, space="PSUM") as pdps,
                tc.tile_pool(name="pd_po", bufs=2, space="PSUM") as pdpo,
                tc.tile_pool(name="pd_pb", bufs=2, space="PSUM") as pdpb,
            ):
                def emit_scores(h, probs):
                    po = (h % 2) * 64
                    hc = h // 2
                    for tt in range(8):
                        ps = pdps.tile([128, 1024], f32, tag="psc")
                        for sh in range(2):
                            nc.tensor.matmul(
                                ps[:, sh * 512:(sh + 1) * 512],
                                nkT[po:po + 64, hc, tt * 128:(tt + 1) * 128],
                                nqT[po:po + 64, hc, sh * 512:(sh + 1) * 512],
                                start=True, stop=True,
                            )
                        nc.scalar.activation(
                            probs[:, tt, :], ps[:], AF.Exp, scale=1.0 / 8.0
                        )

                def emit_out(h, probs):
                    stage = pds.tile([65, SQ], f32, tag="stage")
                    for sh in range(2):
                        ps = pdpo.tile([65, 512], f32, tag="po")
                        for tt in range(8):
                            nc.tensor.matmul(
                                ps[:], v_aug[:, tt, h, :],
                                probs[:, tt, sh * 512:(sh + 1) * 512],
                                start=(tt == 0), stop=(tt == 7),
                            )
                        nc.vector.tensor_copy(
                            stage[:, sh * 512:(sh + 1) * 512], ps[:]
                        )
                    # sums row sits on partition 64; bounce to partition 0
                    # (reciprocal_approx_fast misbehaves off partition 0)
                    nc.sync.dma_start(d_rs2[h:h + 1, :], stage[64:65, :])
                    sums0 = pds.tile([1, SQ], f32, tag="sums0")
                    nc.sync.dma_start(sums0[:], d_rs2[h:h + 1, :])
                    rinv = pds.tile([1, SQ], f32, tag="rinv")
                    nc.vector.reciprocal_approx_fast(rinv[:], sums0[:])
                    rinv_r = pds.tile([1, SQ], f32r, tag="rinv_r")
                    nc.vector.tensor_copy(rinv_r[:], rinv[:])
                    outF = pds.tile([64, SQ], f32, tag="outF")
                    for sh in range(2):
                        shs = slice(sh * 512, (sh + 1) * 512)
                        psb = pdpb.tile([64, 512], f32, tag="pb")
                        nc.tensor.matmul(
                            psb[:], ones_row[:, 0:64], rinv_r[:, shs],
                            start=True, stop=True,
                        )
                        nc.vector.tensor_mul(outF[:, shs], stage[0:64, shs], psb[:])
                    nc.sync.dma_start(d_outT[h * DH:(h + 1) * DH, :], outF[:])

                for hp in range(H // 2):
                    h0, h1 = 2 * hp, 2 * hp + 1
                    probs0 = pdp.tile([128, 8, SQ], bf16, tag="probs_h")
                    probs1 = pdp.tile([128, 8, SQ], bf16, tag="probs_h")
                    emit_scores(h0, probs0)
                    emit_scores(h1, probs1)
                    emit_out(h0, probs0)
                    emit_out(h1, probs1)

    nc.compile()
    return nc


def _prep_in_maps(inputs):
    import ml_dtypes

    bf = ml_dtypes.bfloat16
    w = {
        "WqT": np.ascontiguousarray(np.asarray(inputs["Wq"]).T).astype(bf),
        "WkT": np.ascontiguousarray(np.asarray(inputs["Wk"]).T).astype(bf),
        "WvT": np.ascontiguousarray(np.asarray(inputs["Wv"]).T).astype(bf),
        "WobsT": np.ascontiguousarray(np.asarray(inputs["Wobs"]).T).astype(bf),
        "WmatT": np.ascontiguousarray(np.asarray(inputs["Wmat"]).T).astype(bf),
    }
    vecs = {
        k: np.ascontiguousarray(np.asarray(inputs[k], dtype=np.float32))
        for k in ("bq", "bk", "bv", "bobs", "bmat")
    }
    vecs["ln_g_bf"] = np.asarray(inputs["ln_g"], dtype=np.float32).astype(bf)
    vecs["ln_b_bf"] = np.asarray(inputs["ln_b"], dtype=np.float32).astype(bf)
    hs = np.asarray(inputs["hidden_states"])
    ctx = np.asarray(inputs["context"])
    in_maps = []
    for b in range(N_CORES):
        m = {
            "hsT": np.ascontiguousarray(hs[b].T).astype(bf),
            "ctxT": np.ascontiguousarray(ctx[b].T).astype(bf),
        }
        m.update(w)
        m.update(vecs)
        in_maps.append(m)
    return in_maps


def kernel(hidden_states, context, Wq, bq, Wk, bk, Wv, bv,
           Wobs, bobs, Wmat, bmat, ln_g, ln_b):
    from concourse import bass_utils

    if "nc" not in _CACHE:
        _CACHE["nc"] = _build()
    nc = _CACHE["nc"]

    in_maps = _prep_in_maps(dict(
        hidden_states=hidden_states, context=context, Wq=Wq, Wk=Wk, Wv=Wv,
        Wobs=Wobs, Wmat=Wmat, bq=bq, bk=bk, bv=bv, bobs=bobs, bmat=bmat,
        ln_g=ln_g, ln_b=ln_b,
    ))
    res = bass_utils.run_bass_kernel_spmd(nc, in_maps, core_ids=list(range(N_CORES)))
    out = np.stack([res.results[b]["outT"].T for b in range(N_CORES)], axis=0)
    return out.astype(np.float32)
